# revision 26
# baseline (speedup 1.0000x reference)
"""AttentionHeadCheb distributed Trainium2 kernel (8 NeuronCores).

Destination-node sharding, gather-free main path: host ships xg (x columns
reordered by edge, block-major grouped layout). Device computes per-edge
wx = W01.T @ xg on PE, ar-scores via fused ARW = W@w_right stationaries,
al via one grouped ap_gather from the resident local al table. Segment
softmax via masked scans; denominator and message readouts use bf16
pair-tables with host-forced odd end parity (x8 group padding). No
collective, no remote tables.
"""

import numpy as np
import ml_dtypes

BF16 = ml_dtypes.bfloat16

N_NODES = 50000
IN_DIM = 128
OUT_DIM = 64
NC = 8
NLOC = N_NODES // NC          # 6250
W4 = 8
SEG = 4096                    # packing unit (8 reserved pad slots at start)
RES = 8                       # reserved pad slots per seg
BLK = 16384                   # slots per block (= partition group)
NBLK = 8
TPH = BLK * NBLK              # 131072 slots total
NPART = TPH // W4             # 16384 partials (8-wide reduce)
NLE = 6256                    # NLOC padded to x16
BIAS_PAD = -60.0


def _pack_weights(W_transform, w_left, w_right, W_residual):
    W01 = np.concatenate([W_transform[0], W_transform[1]], axis=1)
    LAL = np.zeros((128, 128), np.float32)
    for i in range(3):
        LAL[0:64, i::16] = w_left[0][i][:, None]
        LAL[64:128, (4 + i)::16] = w_left[1][i][:, None]
    # ARW[:, 4k+i] = W_transform[k] @ w_right[k][i]  (fused x->ar map)
    ARW = np.zeros((128, 16), np.float32)
    for k in range(2):
        for i in range(3):
            ARW[:, 4 * k + i] = W_transform[k] @ w_right[k][i]
    WRT = W_residual[0:IN_DIM]
    WRB = np.concatenate([W_residual[IN_DIM:], W_residual[IN_DIM:]], axis=0)
    DSEL = np.zeros((128, 2), np.float32)
    DSEL[0::16, 0] = 1.0
    DSEL[4::16, 1] = 1.0
    CSEL = np.zeros((128, 128), np.float32)
    for g in range(8):
        for k in range(2):
            CSEL[16 * g + 4 * k:16 * g + 4 * k + 4, 16 * g + 4 * k] = 1.0
    SELALL = np.zeros((128, 1024), np.float32)
    for t in range(8):
        SELALL[16 * t, 128 * t:128 * t + 64] = 1.0
        SELALL[16 * t + 4, 128 * t + 64:128 * t + 128] = 1.0
    DSEL2 = np.zeros((2, 128), np.float32)
    DSEL2[0, 0:64] = 1.0
    DSEL2[1, 64:128] = 1.0
    # SELARW[:, 64g:64g+64]: ARW cols placed at 16*(g%4).. within the
    # 64-partition half so 4 group-matmuls accumulate into one psum half
    SELARW = np.zeros((128, 512), np.float32)
    for g in range(8):
        SELARW[:, 64 * g + 16 * (g % 4):64 * g + 16 * (g % 4) + 16] = ARW
    return (W01.astype(BF16), LAL.astype(BF16), SELARW.astype(BF16),
            WRT.astype(BF16), WRB.astype(BF16), DSEL.astype(BF16),
            CSEL.astype(BF16), SELALL.astype(BF16), DSEL2.astype(BF16))


def _wrap16_rep(vals, nidx):
    v = vals.reshape(nidx // 16, 16).T
    return np.tile(v, (8, 1)).astype(np.int16)


def _wrap16_grouped(vals):
    g, eb = vals.shape
    out = np.empty((16 * g, eb // 16), np.int16)
    for gg in range(g):
        out[16 * gg:16 * gg + 16] = vals[gg].reshape(eb // 16, 16).T
    return out


def _prep_core(m, r, c, atten_vals, support_vals, x_bfT):
    sel = np.where((r >= m * NLOC) & (r < (m + 1) * NLOC))[0]
    rl = (r[sel] - m * NLOC).astype(np.int64)
    order = np.argsort(rl, kind='stable')
    sel, rl = sel[order], rl[order]
    cg = c[sel].astype(np.int64)

    ne = rl.size
    gstart = np.flatnonzero(np.r_[True, rl[1:] != rl[:-1]])
    gcnt = np.diff(np.r_[gstart, ne])
    grow = rl[gstart]
    gpad = ((gcnt + 7) // 8) * 8          # x8 pad -> end slot odd at /4
    ng = grow.size
    gpos = np.empty(ng, np.int64)
    seg_i, off = 0, RES
    NSEG = TPH // SEG
    for i in range(ng):
        if off + gpad[i] > SEG:
            seg_i += 1
            off = RES
        assert seg_i < NSEG, f"core {m}: seg overflow"
        gpos[i] = seg_i * SEG + off
        off += gpad[i]
    within = np.arange(ne) - np.repeat(gstart, gcnt)
    slot = np.repeat(gpos, gcnt) + within
    cols = np.zeros(TPH, np.int64)
    rows = np.zeros(TPH, np.int64)
    vrow = np.zeros((8, TPH), np.float32)
    vrow[3] = BIAS_PAD
    vrow[7] = BIAS_PAD
    cols[slot] = cg
    rows[slot] = rl
    e0 = sel
    vrow[0][slot] = atten_vals[0][e0]
    vrow[1][slot] = atten_vals[1][e0]
    vrow[2][slot] = support_vals[0][e0]
    vrow[3][slot] = 0.0
    vrow[4][slot] = atten_vals[0][e0]
    vrow[5][slot] = atten_vals[1][e0]
    vrow[6][slot] = support_vals[1][e0]
    vrow[7][slot] = 0.0
    esid = np.zeros(TPH, np.int64)
    for si in range(NSEG):
        esid[si * SEG:(si + 1) * SEG] = -(si + 1)
    gp_hi = gpos + gpad
    for i in range(ng):
        esid[gpos[i]:gp_hi[i]] = i
    emask = np.ones(TPH, np.float32)
    emask[0] = 0.0
    emask[1:][esid[1:] != esid[:-1]] = 0.0
    emask[0::SEG] = 0.0
    psid = esid[0::W4]
    pmask = np.ones(TPH // W4, np.float32)
    pmask[0] = 0.0
    pmask[1:][psid[1:] != psid[:-1]] = 0.0
    pmask[0::SEG // W4] = 0.0
    # message readout: single full-phase pair table; parity per row
    pend = gp_hi // W4 - 1
    endp = np.zeros(NLE, np.int64)
    parity = np.zeros(NLE, np.float32)
    for i in range(ng):
        endp[grow[i]] = pend[i] >> 1
        parity[grow[i]] = pend[i] & 1
    # denom readout: block-local end edge pair idx (end edge = 3 mod 4)
    eloc = (gp_hi - 1) % BLK
    gblk = gpos // BLK
    dendp = np.zeros((8, NLE), np.int64)
    for i in range(ng):
        dendp[gblk[i], grow[i]] = eloc[i] >> 1
    rloc = _wrap16_grouped(rows.reshape(8, BLK))
    rloc8 = _wrap16_grouped(np.ascontiguousarray(rows.reshape(8, BLK)[:, ::8]))
    endpw = _wrap16_rep(endp, NLE)
    parw = np.broadcast_to(parity.astype(BF16)[None, :], (128, NLE)).copy()
    dendpw = _wrap16_grouped(dendp)
    vst = np.zeros((128, BLK), BF16)
    for g in range(8):
        for i in range(8):
            vst[16 * g + i] = vrow[i][g * BLK:(g + 1) * BLK].astype(BF16)
    pmrep = np.broadcast_to(pmask.astype(BF16)[None, :],
                            (128, TPH // W4)).copy()
    emrep = np.repeat(emask.reshape(8, BLK).astype(BF16), 16, axis=0)
    xg = np.ascontiguousarray(x_bfT[:, cols])
    return dict(rloc=rloc, rloc8=rloc8, vst=vst, pmrep=pmrep, emrep=emrep, xg=xg,
                endp=endpw, parw=parw, dendp=dendpw,
                emask=emask, pmask=pmask, cols=cols, rows=rows,
                esid=esid)


def host_prep(x, support_vals, atten_vals, W_transform, w_left, w_right,
              W_residual, edge_rows, edge_cols):
    (W01, LAL, SELARW, WRT, WRB, DSEL, CSEL, SELALL,
     DSEL2) = _pack_weights(W_transform, w_left, w_right, W_residual)
    ONESROW = np.ones((1, NLOC), np.float32)
    x_bfT = np.ascontiguousarray(x.T.astype(BF16))
    in_maps = []
    for m in range(NC):
        ph = _prep_core(m, edge_rows, edge_cols, atten_vals, support_vals,
                        x_bfT)
        xT = np.ascontiguousarray(x[m * NLOC:(m + 1) * NLOC].T).astype(BF16)
        im = dict(xT=xT, W01=W01, LAL=LAL, ARW=SELARW, WRT=WRT, WRB=WRB,
                  DSEL=DSEL, CSEL=CSEL, SELALL=SELALL, DSEL2=DSEL2,
                  ONESROW=ONESROW)
        for k in ("rloc", "rloc8", "vst", "pmrep", "emrep", "xg", "endp",
                  "parw", "dendp"):
            im[k] = np.ascontiguousarray(ph[k])
        im["_dbg"] = {k: ph[k] for k in ("emask", "pmask", "cols", "rows",
                                         "esid")}
        in_maps.append(im)
    return in_maps


# ======================================================================
# Numpy emulation (bf16-faithful where it matters)
# ======================================================================

def emulate(in_maps, x, W_transform, w_left, w_right, W_residual):
    xb = x.astype(BF16).astype(np.float32)
    W01 = np.concatenate([W_transform[0], W_transform[1]],
                         axis=1).astype(BF16).astype(np.float32)
    ARW = np.zeros((128, 16), np.float32)
    for k in range(2):
        for i in range(3):
            ARW[:, 4 * k + i] = W_transform[k] @ w_right[k][i]
    ARW = ARW.astype(BF16).astype(np.float32)

    def segscan(parts, mrow):
        cs = np.cumsum(parts.astype(np.float64), axis=-1)
        starts = np.flatnonzero(mrow == 0.0)
        seg = np.cumsum(mrow == 0.0) - 1
        offs = np.take(cs[..., starts] - parts[..., starts], seg, axis=-1)
        return (cs - offs).astype(np.float32)

    outs = []
    for m in range(NC):
        im = in_maps[m]
        dbg = im["_dbg"]
        cols, rows, emask, pmask = (dbg["cols"], dbg["rows"], dbg["emask"],
                                    dbg["pmask"])
        xg = im["xg"].astype(np.float32)          # [128, TPH]
        # al table (local)
        wx_loc = xb[m * NLOC:(m + 1) * NLOC] @ W01   # [NLOC, 128]
        al8 = np.zeros((8, NLOC), np.float32)
        for k in range(2):
            al8[4 * k:4 * k + 3] = (
                wx_loc[:, 64 * k:64 * k + 64] @ w_left[k].T).T
        al8[3] = 1.0
        al8[7] = 1.0
        # scores per slot
        arv = (ARW.T @ xg)                        # [16, TPH] (rows 4k+i)
        vr = np.zeros((8, TPH), np.float32)
        for g in range(8):
            for i in range(8):
                vr[i][g * BLK:(g + 1) * BLK] = im["vst"][
                    16 * g + i].astype(np.float32)
        alv = al8[:, rows]                        # [8, TPH]
        s = np.zeros((2, TPH), np.float32)
        for k in range(2):
            p = (alv[4 * k:4 * k + 4] + np.vstack(
                [arv[4 * k:4 * k + 3], np.zeros(TPH)])) * vr[4 * k:4 * k + 4]
            s[k] = p.sum(0)
        ex = np.exp(s).astype(BF16).astype(np.float32)   # [2, TPH]
        # denom: per-edge segscan + block-pair readout
        dsum = np.zeros((2, NLE), np.float32)
        exs = np.stack([segscan(ex[k], emask) for k in range(2)])
        dd = im["dendp"].astype(np.int64)
        for g in range(8):
            idx = dd[16 * g:16 * g + 16].T.reshape(-1)[:NLE]
            val = exs[:, g * BLK + 2 * idx + 1]
            dsum[0] += val[0]
            dsum[1] += val[1]
        # messages: wxe * ex, partials, scan, pair readout
        wxe = W01.T @ xg                          # [128, TPH]
        exrep = np.repeat(ex, 64, axis=0)         # [128, TPH]
        gb = (wxe * exrep).astype(BF16).astype(np.float32)
        part = gb.reshape(128, NPART, W4).sum(2).astype(BF16).astype(
            np.float32)
        sc = segscan(part, pmask).astype(BF16).astype(np.float32)
        ep = im["endp"].astype(np.int64)
        idx = ep[0:16].T.reshape(-1)[:NLE]
        par = im["parw"][0].astype(np.float32)
        p0 = sc[:, 2 * idx]
        p1 = sc[:, 2 * idx + 1]
        msum = p0 + (p1 - p0) * par
        dsum += 1e-8
        out01 = msum[:, :NLOC].copy()
        out01[0:64] /= dsum[0][:NLOC]
        out01[64:128] /= dsum[1][:NLOC]
        xs = xb[m * NLOC:(m + 1) * NLOC]
        pre = (xs @ W_residual[:IN_DIM] +
               (out01[0:64] + out01[64:128]).T @ W_residual[IN_DIM:])
        out = np.where(pre > 0, pre, np.exp(np.minimum(pre, 0)) - 1)
        outs.append(out.astype(np.float32))
    return np.concatenate(outs, axis=0)


# ======================================================================
# Bass kernel builder
# ======================================================================

def build_bass():
    import sys
    if '/opt/trn_rl_repo' not in sys.path:
        sys.path.insert(0, '/opt/trn_rl_repo')
    from concourse import bass, bacc, tile, mybir

    dt = mybir.dt
    AL = mybir.AluOpType
    AF = mybir.ActivationFunctionType
    AX = mybir.AxisListType

    nc = bacc.Bacc(None, target_bir_lowering=False)

    def din(name, shape, d):
        return nc.dram_tensor(name, list(shape), d, kind="ExternalInput")

    xT_d = din("xT", (128, NLOC), dt.bfloat16)
    W01_d = din("W01", (128, 128), dt.bfloat16)
    LAL_d = din("LAL", (128, 128), dt.bfloat16)
    ARW_d = din("ARW", (128, 512), dt.bfloat16)
    WRT_d = din("WRT", (128, 64), dt.bfloat16)
    WRB_d = din("WRB", (128, 64), dt.bfloat16)
    DSEL_d = din("DSEL", (128, 2), dt.bfloat16)
    CSEL_d = din("CSEL", (128, 128), dt.bfloat16)
    SELALL_d = din("SELALL", (128, 1024), dt.bfloat16)
    DSEL2_d = din("DSEL2", (2, 128), dt.bfloat16)
    ONESROW_d = din("ONESROW", (1, NLOC), dt.float32)
    rloc8_d = din("rloc8", (128, BLK // 128), dt.int16)
    vst_d = din("vst", (128, BLK), dt.bfloat16)
    pmrep_d = din("pmrep", (128, NPART), dt.bfloat16)
    emrep_d = din("emrep", (128, BLK), dt.bfloat16)
    xg_d = din("xg", (128, TPH), dt.bfloat16)
    endp_d = din("endp", (128, NLE // 16), dt.int16)
    parw_d = din("parw", (128, NLE), dt.bfloat16)
    dendp_d = din("dendp", (128, NLE // 16), dt.int16)
    out_d = nc.dram_tensor("out", [64, NLOC], dt.float32,
                           kind="ExternalOutput")

    NT512 = (NLOC + 511) // 512
    NJ1K = (NLE + 1023) // 1024

    with tile.TileContext(nc) as tc:
      with nc.allow_low_precision(reason="bf16 accums validated in emulation"):
        with (
            tc.tile_pool(name="res", bufs=1) as res,
            tc.tile_pool(name="mid", bufs=1) as mid,
            tc.tile_pool(name="work", bufs=2) as work,
            tc.tile_pool(name="psum", bufs=3, space="PSUM") as psum,
        ):
            # ---------- stage 1: local wx -> al table ----------
            xT = res.tile([128, BLK], dt.bfloat16, tag="sc32a", name="xT")
            nc.sync.dma_start(xT[:, 0:NLOC], xT_d[:])
            W01 = mid.tile([128, 128], dt.bfloat16, tag="w128")
            nc.sync.dma_start(W01[:], W01_d[:])
            LALt = mid.tile([128, 128], dt.bfloat16, tag="w128b")
            nc.sync.dma_start(LALt[:], LAL_d[:])
            al8r = res.tile([128, NLOC], dt.float32, tag="al8r")
            wxb = res.tile([128, NLOC], dt.bfloat16, tag="sc32b",
                           name="wxb")
            for j in range(NT512):
                a, b = j * 512, min(NLOC, (j + 1) * 512)
                pw = psum.tile([128, 1024], dt.float32, tag="p4k",
                               name="pw")
                nc.tensor.matmul(pw[:, :b - a], W01[:], xT[:, a:b],
                                 start=True, stop=True)
                nc.scalar.activation(wxb[:, a:b], pw[:, :b - a], AF.Copy)
            for j in range(NT512):
                a, b = j * 512, min(NLOC, (j + 1) * 512)
                pa = psum.tile([128, 1024], dt.float32, tag="p4k",
                               name="pa")
                nc.tensor.matmul(pa[:, :b - a], LALt[:], wxb[:, a:b],
                                 start=True, stop=True)
                nc.scalar.activation(al8r[:, a:b], pa[:, :b - a], AF.Copy)
            for g8 in range(8):
                nc.scalar.dma_start(al8r[16 * g8 + 3:16 * g8 + 4, :],
                                    ONESROW_d[:])
                nc.scalar.dma_start(al8r[16 * g8 + 7:16 * g8 + 8, :],
                                    ONESROW_d[:])

            msum = res.tile([128, NLE], dt.bfloat16, tag="msum")
            dsum = mid.tile([2, NLE], dt.bfloat16, tag="dsum")
            nc.vector.memset(dsum[:], 0.0)
            nc.vector.memset(msum[:], 0.0)
            DSELt = mid.tile([128, 2], dt.bfloat16, tag="dsel")
            nc.sync.dma_start(DSELt[:], DSEL_d[:])
            CSELt = mid.tile([128, 128], dt.bfloat16, tag="csel")
            nc.sync.dma_start(CSELt[:], CSEL_d[:])
            SELt = mid.tile([128, 1024], dt.bfloat16, tag="selall")
            nc.sync.dma_start(SELt[:], SELALL_d[:])
            DSEL2t = mid.tile([2, 128], dt.bfloat16, tag="dsel2")
            nc.sync.dma_start(DSEL2t[:], DSEL2_d[:])
            ARWt = mid.tile([128, 512], dt.bfloat16, tag="arw")
            nc.sync.dma_start(ARWt[:], ARW_d[:])
            rloc8 = mid.tile([128, BLK // 128], dt.int16, tag="rloc")
            nc.sync.dma_start(rloc8[:], rloc8_d[:])

            # ---------- B: scores -> ex8 (grouped layout) ----------
            alo8 = mid.tile([128, BLK // 8], dt.float32, tag="alo8")
            for ah in range(2):
                aw = BLK // 16 // 16
                nc.gpsimd.ap_gather(alo8[:, ah * (BLK // 16):
                                         (ah + 1) * (BLK // 16)],
                                    al8r[:],
                                    rloc8[:, ah * aw:(ah + 1) * aw],
                                    channels=128, num_elems=NLOC, d=1,
                                    num_idxs=BLK // 16)
            ex8 = res.tile([128, BLK], dt.bfloat16, tag="ex8")
            for cch in range(16):
                a, b = cch * 1024, (cch + 1) * 1024
                xgb = work.tile([128, 8, 1024], dt.bfloat16, tag="xgb",
                                bufs=1)
                for g in range(8):
                    nc.sync.dma_start(
                        xgb[:, g, :], xg_d[:, g * BLK + a:g * BLK + b])
                vsts = work.tile([128, 1024], dt.bfloat16, tag="vsts",
                                 bufs=1)
                nc.sync.dma_start(vsts[:], vst_d[:, a:b])

                arv = psum.tile([128, 1024], dt.float32, tag="p4k",
                                name="arv")
                for half in (0, 1):
                    for gg in range(4):
                        g = 4 * half + gg
                        sel = ARWt[:, 64 * g:64 * g + 64]
                        for v2 in range(2):
                            va = v2 * 512
                            nc.tensor.matmul(
                                arv[64 * half:64 * half + 64, va:va + 512],
                                sel, xgb[:, g, va:va + 512],
                                start=(gg == 0), stop=(gg == 3))
                p8 = work.tile([128, 1024], dt.bfloat16, tag="b2k",
                               name="p8")
                albc = alo8[:, cch * 128:(cch + 1) * 128].rearrange(
                    "p (a b) -> p a b", b=1).broadcast_to([128, 128, 8])
                arvv = arv[:].rearrange("p (a b) -> p a b", b=8)
                p8v = p8[:].rearrange("p (a b) -> p a b", b=8)
                nc.vector.tensor_tensor(p8v, albc, arvv, AL.add)
                nc.vector.tensor_tensor(p8[:], p8[:], vsts[:], AL.mult)
                sxp = psum.tile([128, 1024], dt.float32, tag="p4k",
                                name="sxp")
                nc.tensor.matmul(sxp[:, 0:512], CSELt[:], p8[:, 0:512],
                                 start=True, stop=True)
                nc.tensor.matmul(sxp[:, 512:1024], CSELt[:],
                                 p8[:, 512:1024], start=True, stop=True)
                nc.scalar.activation(ex8[:, a:b], sxp[:], AF.Exp)

            # ---------- denom: edge scans + pair readout ----------
            exs = res.tile([128, BLK], dt.bfloat16, tag="sc32b",
                           name="exs")
            for sq in range(4):
                a, b = sq * SEG, (sq + 1) * SEG
                emaskh = mid.tile([128, 4096], dt.bfloat16, tag="mask8k",
                                  name="emaskh")
                nc.sync.dma_start(emaskh[:], emrep_d[:, a:b])
                nc.vector.tensor_tensor_scan(
                    exs[:, a:b], emaskh[:], ex8[:, a:b], 0.0,
                    op0=AL.mult, op1=AL.add)
            dendw = mid.tile([128, NLE // 16], dt.int16, tag="endw",
                             bufs=3, name="dendw")
            nc.sync.dma_start(dendw[:], dendp_d[:])
            exsp = exs[:].rearrange("p (a b) -> p a b", b=2)
            for j in range(NJ1K):
                a, b = j * 1024, min(NLE, (j + 1) * 1024)
                dgp = work.tile([128, 1024, 2], dt.bfloat16, tag="g4k",
                                name="dgp")
                nc.gpsimd.ap_gather(
                    dgp[:, :b - a, :], exsp,
                    dendw[:, a // 16:(a + (b - a)) // 16],
                    channels=128, num_elems=BLK // 2, d=2,
                    num_idxs=b - a)
                pdn = psum.tile([2, 1024], dt.float32, tag="pdn",
                                name="pdn", bufs=1)
                for va in range(0, b - a, 512):
                    vb = min(b - a, va + 512)
                    nc.tensor.matmul(pdn[:, va:vb], DSELt[:],
                                     dgp[:, va:vb, 1],
                                     start=True, stop=True)
                nc.vector.tensor_tensor(dsum[:, a:b], dsum[:, a:b],
                                        pdn[:, :b - a], AL.add)

            nc.vector.tensor_scalar(dsum[:], dsum[:], 1e-8, None, AL.add)
            drec = dsum
            nc.vector.reciprocal(drec[:], dsum[:])

            # ---------- C: messages, single pass ----------
            pp = res.tile([128, NPART], dt.bfloat16, tag="sc32a",
                          name="pp")
            for s in range(32):
                # subiter covers 4096 slots = 512 partials
                s0 = s * 4096
                t = s0 // BLK
                e0 = s0 % BLK
                for cc in range(4):
                    c0 = s0 + cc * 1024
                    ce = e0 + cc * 1024
                    xgc = work.tile([128, 1024], dt.bfloat16,
                                    tag="xgc")
                    nc.sync.dma_start(xgc[:], xg_d[:, c0:c0 + 1024])
                    wxe = psum.tile([128, 1024], dt.float32, tag="p4k",
                                    name="wxe")
                    nc.tensor.matmul(wxe[:, 0:512], W01[:],
                                     xgc[:, 0:512],
                                     start=True, stop=True)
                    nc.tensor.matmul(wxe[:, 512:1024], W01[:],
                                     xgc[:, 512:1024],
                                     start=True, stop=True)
                    gtS = work.tile([128, 1024], dt.bfloat16,
                                    tag="b2k", name="gtS")
                    nc.scalar.activation(gtS[:], wxe[:], AF.Copy)
                    exrep = psum.tile([128, 1024], dt.float32,
                                      tag="p4k", name="exrep")
                    sel = SELt[:, t * 128:(t + 1) * 128]
                    nc.tensor.matmul(exrep[:, 0:512], sel,
                                     ex8[:, ce:ce + 512],
                                     start=True, stop=True)
                    nc.tensor.matmul(exrep[:, 512:1024], sel,
                                     ex8[:, ce + 512:ce + 1024],
                                     start=True, stop=True)
                    gb = work.tile([128, 128, 8], dt.bfloat16,
                                   tag="gb")
                    g2o = gb[:].rearrange("p a b -> p (a b)")
                    nc.vector.tensor_tensor(g2o[:, :], gtS[:],
                                            exrep[:], AL.mult)
                    pb = s * 512 + cc * 128
                    if s >= 16:
                        # halve on gpsimd in place, then 4-reduce on DVE
                        nc.gpsimd.tensor_tensor(
                            gb[:, :, 0:4], gb[:, :, 0:4], gb[:, :, 4:8],
                            AL.add)
                        nc.vector.tensor_reduce(
                            pp[:, pb:pb + 128], gb[:, :, 0:4], axis=AX.X,
                            op=AL.add)
                    else:
                        nc.vector.tensor_reduce(
                            pp[:, pb:pb + 128], gb[:], axis=AX.X,
                            op=AL.add)
            ppscan = res.tile([128, NPART], dt.bfloat16, tag="sc32b",
                              name="ppscan")
            for sq in range(4):
                a, b = sq * 4096, (sq + 1) * 4096
                pmq = mid.tile([128, 4096], dt.bfloat16, tag="mask8k",
                               name="pmq")
                nc.sync.dma_start(pmq[:], pmrep_d[:, a:b])
                nc.vector.tensor_tensor_scan(
                    ppscan[:, a:b], pmq[:], pp[:, a:b], 0.0,
                    op0=AL.mult, op1=AL.add)
            endw = mid.tile([128, NLE // 16], dt.int16, tag="endw",
                            bufs=3, name="endw")
            nc.sync.dma_start(endw[:], endp_d[:])
            scp = ppscan[:].rearrange("p (a b) -> p a b", b=2)
            msb = res.tile([128, NLOC], dt.bfloat16, tag="sc32b",
                           name="msb")
            xTr = res.tile([128, NLOC], dt.bfloat16, tag="sc32a",
                           name="xTr")
            nc.sync.dma_start(xTr[:], xT_d[:])
            WRTt = mid.tile([128, 64], dt.bfloat16, tag="w128")
            WRBt = mid.tile([128, 64], dt.bfloat16, tag="w128b")
            nc.sync.dma_start(WRTt[:], WRT_d[:])
            nc.sync.dma_start(WRBt[:], WRB_d[:])
            osb = res.tile([64, NLOC], dt.float32, tag="ex8", name="osb")
            for j in range(NJ1K):
                a, b = j * 1024, min(NLE, (j + 1) * 1024)
                ehp = work.tile([128, 1024, 2], dt.bfloat16,
                                tag="g4k", name="ehp")
                nc.gpsimd.ap_gather(
                    ehp[:, :b - a, :], scp,
                    endw[:, a // 16:(a + (b - a)) // 16],
                    channels=128, num_elems=NPART // 2, d=2,
                    num_idxs=b - a)
                parc = work.tile([128, 1024], dt.bfloat16, tag="xgc",
                                 name="parc")
                nc.sync.dma_start(parc[:, :b - a], parw_d[:, a:b])
                pdf = work.tile([128, 1024], dt.bfloat16, tag="b2k",
                                name="pdf")
                nc.vector.tensor_tensor(pdf[:, :b - a],
                                        ehp[:, :b - a, 1],
                                        ehp[:, :b - a, 0], AL.subtract)
                nc.vector.tensor_tensor(pdf[:, :b - a], pdf[:, :b - a],
                                        parc[:, :b - a], AL.mult)
                nc.vector.tensor_tensor(
                    msum[:, a:b], msum[:, a:b],
                    ehp[:, :b - a, 0], AL.add)
                nc.vector.tensor_tensor(
                    msum[:, a:b], msum[:, a:b],
                    pdf[:, :b - a], AL.add)
                # stage-4 for the finished column range, under the next
                # gather's latency
                for a2 in range(a, min(b, NLOC), 512):
                    b2 = min(a2 + 512, NLOC)
                    drep = psum.tile([128, 1024], dt.float32, tag="p4k",
                                     name="drep")
                    nc.tensor.matmul(drep[:, :b2 - a2], DSEL2t[:],
                                     drec[:, a2:b2], start=True, stop=True)
                    nc.vector.tensor_tensor(msb[:, a2:b2],
                                            msum[:, a2:b2],
                                            drep[:, :b2 - a2], AL.mult)
                    prj = psum.tile([64, 1024], dt.float32, tag="pdn",
                                    name="prj", bufs=1)
                    nc.tensor.matmul(prj[:, :b2 - a2], WRTt[:],
                                     xTr[:, a2:b2], start=True, stop=False)
                    nc.tensor.matmul(prj[:, :b2 - a2], WRBt[:],
                                     msb[:, a2:b2], start=False, stop=True)
                    et = work.tile([64, 512], dt.float32, tag="et4k",
                                   bufs=1, name="et")
                    nc.scalar.activation(et[:, :b2 - a2], prj[:, :b2 - a2],
                                         AF.Exp)
                    nc.vector.tensor_scalar(et[:, :b2 - a2],
                                            et[:, :b2 - a2],
                                            -1.0, 0.0, AL.add, AL.min)
                    nc.vector.scalar_tensor_tensor(
                        osb[:, a2:b2], prj[:, :b2 - a2], 0.0,
                        et[:, :b2 - a2], op0=AL.max, op1=AL.add)
            nc.sync.dma_start(out_d[:], osb[:])

    nc.compile()
    return nc


_CACHED = {}


def kernel(**inputs):
    import sys
    if '/opt/trn_rl_repo' not in sys.path:
        sys.path.insert(0, '/opt/trn_rl_repo')
    from concourse import bass_utils

    np_inputs = {k: np.asarray(v) for k, v in inputs.items()}
    in_maps = host_prep(**np_inputs)
    for im in in_maps:
        im.pop("_dbg", None)
    if 'nc' not in _CACHED:
        _CACHED['nc'] = build_bass()
    nc = _CACHED['nc']
    res = bass_utils.run_bass_kernel_spmd(nc, in_maps,
                                          core_ids=list(range(NC)))
    outs = [res.results[m]["out"] for m in range(NC)]
    return np.concatenate([o.T for o in outs], axis=0).astype(np.float32)


# revision 27
# speedup vs baseline: 1.1286x; 1.1286x over previous
"""AttentionHeadCheb distributed Trainium2 kernel (8 NeuronCores).

Destination-node sharding, gather-free main path: host ships xg (x columns
reordered by edge, block-major grouped layout). Device computes per-edge
wx = W01.T @ xg on PE, ar-scores via fused ARW = W@w_right stationaries,
al via one grouped ap_gather from the resident local al table. Segment
softmax via masked scans; denominator and message readouts use bf16
pair-tables with host-forced odd end parity (x8 group padding). No
collective, no remote tables.
"""

import numpy as np
import ml_dtypes

BF16 = ml_dtypes.bfloat16

N_NODES = 50000
IN_DIM = 128
OUT_DIM = 64
NC = 8
NLOC = N_NODES // NC          # 6250
W4 = 8
SEG = 4096                    # packing unit (8 reserved pad slots at start)
RES = 8                       # reserved pad slots per seg
BLK = 16384                   # slots per block (= partition group)
NBLK = 8
TPH = BLK * NBLK              # 131072 slots total
NPART = TPH // W4             # 16384 partials (8-wide reduce)
NLE = 6256                    # NLOC padded to x16
BIAS_PAD = -60.0


def _pack_weights(W_transform, w_left, w_right, W_residual):
    W01 = np.concatenate([W_transform[0], W_transform[1]], axis=1)
    LAL = np.zeros((128, 128), np.float32)
    for i in range(3):
        LAL[0:64, i::16] = w_left[0][i][:, None]
        LAL[64:128, (4 + i)::16] = w_left[1][i][:, None]
    # ARW[:, 4k+i] = W_transform[k] @ w_right[k][i]  (fused x->ar map)
    ARW = np.zeros((128, 16), np.float32)
    for k in range(2):
        for i in range(3):
            ARW[:, 4 * k + i] = W_transform[k] @ w_right[k][i]
    WRT = W_residual[0:IN_DIM]
    WRB = np.concatenate([W_residual[IN_DIM:], W_residual[IN_DIM:]], axis=0)
    DSEL = np.zeros((128, 2), np.float32)
    DSEL[0::16, 0] = 1.0
    DSEL[4::16, 1] = 1.0
    CSEL = np.zeros((128, 128), np.float32)
    for g in range(8):
        for k in range(2):
            CSEL[16 * g + 4 * k:16 * g + 4 * k + 4, 16 * g + 4 * k] = 1.0
    SELALL = np.zeros((128, 1024), np.float32)
    for t in range(8):
        SELALL[16 * t, 128 * t:128 * t + 64] = 1.0
        SELALL[16 * t + 4, 128 * t + 64:128 * t + 128] = 1.0
    DSEL2 = np.zeros((2, 128), np.float32)
    DSEL2[0, 0:64] = 1.0
    DSEL2[1, 64:128] = 1.0
    # SELARW[:, 64g:64g+64]: ARW cols placed at 16*(g%4).. within the
    # 64-partition half so 4 group-matmuls accumulate into one psum half
    SELARW = np.zeros((128, 512), np.float32)
    for g in range(8):
        SELARW[:, 64 * g + 16 * (g % 4):64 * g + 16 * (g % 4) + 16] = ARW
    return (W01.astype(BF16), LAL.astype(BF16), SELARW.astype(BF16),
            WRT.astype(BF16), WRB.astype(BF16), DSEL.astype(BF16),
            CSEL.astype(BF16), SELALL.astype(BF16), DSEL2.astype(BF16))


def _wrap16_rep(vals, nidx):
    v = vals.reshape(nidx // 16, 16).T
    return np.tile(v, (8, 1)).astype(np.int16)


def _wrap16_grouped(vals):
    g, eb = vals.shape
    out = np.empty((16 * g, eb // 16), np.int16)
    for gg in range(g):
        out[16 * gg:16 * gg + 16] = vals[gg].reshape(eb // 16, 16).T
    return out


def _prep_core(m, r, c, atten_vals, support_vals, x_bfT):
    sel = np.where((r >= m * NLOC) & (r < (m + 1) * NLOC))[0]
    rl = (r[sel] - m * NLOC).astype(np.int64)
    order = np.argsort(rl, kind='stable')
    sel, rl = sel[order], rl[order]
    cg = c[sel].astype(np.int64)

    ne = rl.size
    gstart = np.flatnonzero(np.r_[True, rl[1:] != rl[:-1]])
    gcnt = np.diff(np.r_[gstart, ne])
    grow = rl[gstart]
    gpad = ((gcnt + 7) // 8) * 8          # x8 pad -> end slot odd at /4
    ng = grow.size
    gpos = np.empty(ng, np.int64)
    seg_i, off = 0, RES
    NSEG = TPH // SEG
    for i in range(ng):
        if off + gpad[i] > SEG:
            seg_i += 1
            off = RES
        assert seg_i < NSEG, f"core {m}: seg overflow"
        gpos[i] = seg_i * SEG + off
        off += gpad[i]
    within = np.arange(ne) - np.repeat(gstart, gcnt)
    slot = np.repeat(gpos, gcnt) + within
    cols = np.zeros(TPH, np.int64)
    rows = np.zeros(TPH, np.int64)
    vrow = np.zeros((8, TPH), np.float32)
    vrow[3] = BIAS_PAD
    vrow[7] = BIAS_PAD
    cols[slot] = cg
    rows[slot] = rl
    e0 = sel
    vrow[0][slot] = atten_vals[0][e0]
    vrow[1][slot] = atten_vals[1][e0]
    vrow[2][slot] = support_vals[0][e0]
    vrow[3][slot] = 0.0
    vrow[4][slot] = atten_vals[0][e0]
    vrow[5][slot] = atten_vals[1][e0]
    vrow[6][slot] = support_vals[1][e0]
    vrow[7][slot] = 0.0
    esid = np.zeros(TPH, np.int64)
    for si in range(NSEG):
        esid[si * SEG:(si + 1) * SEG] = -(si + 1)
    gp_hi = gpos + gpad
    for i in range(ng):
        esid[gpos[i]:gp_hi[i]] = i
    emask = np.ones(TPH, np.float32)
    emask[0] = 0.0
    emask[1:][esid[1:] != esid[:-1]] = 0.0
    emask[0::SEG] = 0.0
    psid = esid[0::W4]
    pmask = np.ones(TPH // W4, np.float32)
    pmask[0] = 0.0
    pmask[1:][psid[1:] != psid[:-1]] = 0.0
    pmask[0::SEG // W4] = 0.0
    # message readout: single full-phase pair table; parity per row
    pend = gp_hi // W4 - 1
    endp = np.zeros(NLE, np.int64)
    parity = np.zeros(NLE, np.float32)
    for i in range(ng):
        endp[grow[i]] = pend[i] >> 1
        parity[grow[i]] = pend[i] & 1
    # denom readout: block-local end edge pair idx (end edge = 3 mod 4)
    eloc = (gp_hi - 1) % BLK
    gblk = gpos // BLK
    dendp = np.zeros((8, NLE), np.int64)
    for i in range(ng):
        dendp[gblk[i], grow[i]] = eloc[i] >> 1
    rloc = _wrap16_grouped(rows.reshape(8, BLK))
    rloc8 = _wrap16_grouped(np.ascontiguousarray(rows.reshape(8, BLK)[:, ::8]))
    endpw = _wrap16_rep(endp, NLE)
    parw = np.broadcast_to(parity.astype(BF16)[None, :], (128, NLE)).copy()
    dendpw = _wrap16_grouped(dendp)
    vst = np.zeros((128, BLK), BF16)
    for g in range(8):
        for i in range(8):
            vst[16 * g + i] = vrow[i][g * BLK:(g + 1) * BLK].astype(BF16)
    pmrep = np.broadcast_to(pmask.astype(BF16)[None, :],
                            (128, TPH // W4)).copy()
    emrep = np.repeat(emask.reshape(8, BLK).astype(BF16), 16, axis=0)
    xg = np.ascontiguousarray(x_bfT[:, cols])
    return dict(rloc=rloc, rloc8=rloc8, vst=vst, pmrep=pmrep, emrep=emrep, xg=xg,
                endp=endpw, parw=parw, dendp=dendpw,
                emask=emask, pmask=pmask, cols=cols, rows=rows,
                esid=esid)


def host_prep(x, support_vals, atten_vals, W_transform, w_left, w_right,
              W_residual, edge_rows, edge_cols):
    (W01, LAL, SELARW, WRT, WRB, DSEL, CSEL, SELALL,
     DSEL2) = _pack_weights(W_transform, w_left, w_right, W_residual)
    ONESROW = np.ones((1, NLOC), np.float32)
    x_bfT = np.ascontiguousarray(x.T.astype(BF16))
    in_maps = []
    for m in range(NC):
        ph = _prep_core(m, edge_rows, edge_cols, atten_vals, support_vals,
                        x_bfT)
        xT = np.ascontiguousarray(x[m * NLOC:(m + 1) * NLOC].T).astype(BF16)
        im = dict(xT=xT, W01=W01, LAL=LAL, ARW=SELARW, WRT=WRT, WRB=WRB,
                  DSEL=DSEL, CSEL=CSEL, SELALL=SELALL, DSEL2=DSEL2,
                  ONESROW=ONESROW)
        for k in ("rloc", "rloc8", "vst", "pmrep", "emrep", "xg", "endp",
                  "parw", "dendp"):
            im[k] = np.ascontiguousarray(ph[k])
        im["_dbg"] = {k: ph[k] for k in ("emask", "pmask", "cols", "rows",
                                         "esid")}
        in_maps.append(im)
    return in_maps


# ======================================================================
# Numpy emulation (bf16-faithful where it matters)
# ======================================================================

def emulate(in_maps, x, W_transform, w_left, w_right, W_residual):
    xb = x.astype(BF16).astype(np.float32)
    W01 = np.concatenate([W_transform[0], W_transform[1]],
                         axis=1).astype(BF16).astype(np.float32)
    ARW = np.zeros((128, 16), np.float32)
    for k in range(2):
        for i in range(3):
            ARW[:, 4 * k + i] = W_transform[k] @ w_right[k][i]
    ARW = ARW.astype(BF16).astype(np.float32)

    def segscan(parts, mrow):
        cs = np.cumsum(parts.astype(np.float64), axis=-1)
        starts = np.flatnonzero(mrow == 0.0)
        seg = np.cumsum(mrow == 0.0) - 1
        offs = np.take(cs[..., starts] - parts[..., starts], seg, axis=-1)
        return (cs - offs).astype(np.float32)

    outs = []
    for m in range(NC):
        im = in_maps[m]
        dbg = im["_dbg"]
        cols, rows, emask, pmask = (dbg["cols"], dbg["rows"], dbg["emask"],
                                    dbg["pmask"])
        xg = im["xg"].astype(np.float32)          # [128, TPH]
        # al table (local)
        wx_loc = xb[m * NLOC:(m + 1) * NLOC] @ W01   # [NLOC, 128]
        al8 = np.zeros((8, NLOC), np.float32)
        for k in range(2):
            al8[4 * k:4 * k + 3] = (
                wx_loc[:, 64 * k:64 * k + 64] @ w_left[k].T).T
        al8[3] = 1.0
        al8[7] = 1.0
        # scores per slot
        arv = (ARW.T @ xg)                        # [16, TPH] (rows 4k+i)
        vr = np.zeros((8, TPH), np.float32)
        for g in range(8):
            for i in range(8):
                vr[i][g * BLK:(g + 1) * BLK] = im["vst"][
                    16 * g + i].astype(np.float32)
        alv = al8[:, rows]                        # [8, TPH]
        s = np.zeros((2, TPH), np.float32)
        for k in range(2):
            p = (alv[4 * k:4 * k + 4] + np.vstack(
                [arv[4 * k:4 * k + 3], np.zeros(TPH)])) * vr[4 * k:4 * k + 4]
            s[k] = p.sum(0)
        ex = np.exp(s).astype(BF16).astype(np.float32)   # [2, TPH]
        # denom: per-edge segscan + block-pair readout
        dsum = np.zeros((2, NLE), np.float32)
        exs = np.stack([segscan(ex[k], emask) for k in range(2)])
        dd = im["dendp"].astype(np.int64)
        for g in range(8):
            idx = dd[16 * g:16 * g + 16].T.reshape(-1)[:NLE]
            val = exs[:, g * BLK + 2 * idx + 1]
            dsum[0] += val[0]
            dsum[1] += val[1]
        # messages: wxe * ex, partials, scan, pair readout
        wxe = W01.T @ xg                          # [128, TPH]
        exrep = np.repeat(ex, 64, axis=0)         # [128, TPH]
        gb = (wxe * exrep).astype(BF16).astype(np.float32)
        part = gb.reshape(128, NPART, W4).sum(2).astype(BF16).astype(
            np.float32)
        sc = segscan(part, pmask).astype(BF16).astype(np.float32)
        ep = im["endp"].astype(np.int64)
        idx = ep[0:16].T.reshape(-1)[:NLE]
        par = im["parw"][0].astype(np.float32)
        p0 = sc[:, 2 * idx]
        p1 = sc[:, 2 * idx + 1]
        msum = p0 + (p1 - p0) * par
        dsum += 1e-8
        out01 = msum[:, :NLOC].copy()
        out01[0:64] /= dsum[0][:NLOC]
        out01[64:128] /= dsum[1][:NLOC]
        xs = xb[m * NLOC:(m + 1) * NLOC]
        pre = (xs @ W_residual[:IN_DIM] +
               (out01[0:64] + out01[64:128]).T @ W_residual[IN_DIM:])
        out = np.where(pre > 0, pre, np.exp(np.minimum(pre, 0)) - 1)
        outs.append(out.astype(np.float32))
    return np.concatenate(outs, axis=0)


# ======================================================================
# Bass kernel builder
# ======================================================================

def build_bass():
    import sys
    if '/opt/trn_rl_repo' not in sys.path:
        sys.path.insert(0, '/opt/trn_rl_repo')
    from concourse import bass, bacc, tile, mybir

    dt = mybir.dt
    AL = mybir.AluOpType
    AF = mybir.ActivationFunctionType
    AX = mybir.AxisListType

    nc = bacc.Bacc(None, target_bir_lowering=False)

    def din(name, shape, d):
        return nc.dram_tensor(name, list(shape), d, kind="ExternalInput")

    xT_d = din("xT", (128, NLOC), dt.bfloat16)
    W01_d = din("W01", (128, 128), dt.bfloat16)
    LAL_d = din("LAL", (128, 128), dt.bfloat16)
    ARW_d = din("ARW", (128, 512), dt.bfloat16)
    WRT_d = din("WRT", (128, 64), dt.bfloat16)
    WRB_d = din("WRB", (128, 64), dt.bfloat16)
    DSEL_d = din("DSEL", (128, 2), dt.bfloat16)
    CSEL_d = din("CSEL", (128, 128), dt.bfloat16)
    SELALL_d = din("SELALL", (128, 1024), dt.bfloat16)
    DSEL2_d = din("DSEL2", (2, 128), dt.bfloat16)
    ONESROW_d = din("ONESROW", (1, NLOC), dt.float32)
    rloc8_d = din("rloc8", (128, BLK // 128), dt.int16)
    vst_d = din("vst", (128, BLK), dt.bfloat16)
    pmrep_d = din("pmrep", (128, NPART), dt.bfloat16)
    emrep_d = din("emrep", (128, BLK), dt.bfloat16)
    xg_d = din("xg", (128, TPH), dt.bfloat16)
    endp_d = din("endp", (128, NLE // 16), dt.int16)
    parw_d = din("parw", (128, NLE), dt.bfloat16)
    dendp_d = din("dendp", (128, NLE // 16), dt.int16)
    out_d = nc.dram_tensor("out", [64, NLOC], dt.float32,
                           kind="ExternalOutput")

    NT512 = (NLOC + 511) // 512
    NJ1K = (NLE + 1023) // 1024

    with tile.TileContext(nc) as tc:
      with nc.allow_low_precision(reason="bf16 accums validated in emulation"):
        with (
            tc.tile_pool(name="res", bufs=1) as res,
            tc.tile_pool(name="mid", bufs=1) as mid,
            tc.tile_pool(name="work", bufs=2) as work,
            tc.tile_pool(name="psum", bufs=3, space="PSUM") as psum,
        ):
            # ---------- stage 1: local wx -> al table ----------
            xT = res.tile([128, BLK], dt.bfloat16, tag="sc32a", name="xT")
            nc.sync.dma_start(xT[:, 0:NLOC], xT_d[:])
            W01 = mid.tile([128, 128], dt.bfloat16, tag="w128")
            nc.sync.dma_start(W01[:], W01_d[:])
            LALt = mid.tile([128, 128], dt.bfloat16, tag="w128b")
            nc.sync.dma_start(LALt[:], LAL_d[:])
            al8r = res.tile([128, NLOC], dt.float32, tag="al8r")
            wxb = res.tile([128, NLOC], dt.bfloat16, tag="sc32b",
                           name="wxb")
            for j in range(NT512):
                a, b = j * 512, min(NLOC, (j + 1) * 512)
                pw = psum.tile([128, 1024], dt.float32, tag="p4k",
                               name="pw")
                nc.tensor.matmul(pw[:, :b - a], W01[:], xT[:, a:b],
                                 start=True, stop=True)
                nc.scalar.activation(wxb[:, a:b], pw[:, :b - a], AF.Copy)
            for j in range(NT512):
                a, b = j * 512, min(NLOC, (j + 1) * 512)
                pa = psum.tile([128, 1024], dt.float32, tag="p4k",
                               name="pa")
                nc.tensor.matmul(pa[:, :b - a], LALt[:], wxb[:, a:b],
                                 start=True, stop=True)
                nc.scalar.activation(al8r[:, a:b], pa[:, :b - a], AF.Copy)
            for g8 in range(8):
                nc.scalar.dma_start(al8r[16 * g8 + 3:16 * g8 + 4, :],
                                    ONESROW_d[:])
                nc.scalar.dma_start(al8r[16 * g8 + 7:16 * g8 + 8, :],
                                    ONESROW_d[:])

            msum = res.tile([128, NLE], dt.bfloat16, tag="msum")
            dsum = mid.tile([2, NLE], dt.bfloat16, tag="dsum")
            nc.vector.memset(dsum[:], 0.0)
            nc.vector.memset(msum[:], 0.0)
            DSELt = mid.tile([128, 2], dt.bfloat16, tag="dsel")
            nc.sync.dma_start(DSELt[:], DSEL_d[:])
            CSELt = mid.tile([128, 128], dt.bfloat16, tag="csel")
            nc.sync.dma_start(CSELt[:], CSEL_d[:])
            SELt = mid.tile([128, 1024], dt.bfloat16, tag="selall")
            nc.sync.dma_start(SELt[:], SELALL_d[:])
            DSEL2t = mid.tile([2, 128], dt.bfloat16, tag="dsel2")
            nc.sync.dma_start(DSEL2t[:], DSEL2_d[:])
            ARWt = mid.tile([128, 512], dt.bfloat16, tag="arw")
            nc.sync.dma_start(ARWt[:], ARW_d[:])
            rloc8 = mid.tile([128, BLK // 128], dt.int16, tag="rloc")
            nc.sync.dma_start(rloc8[:], rloc8_d[:])

            # ---------- B: scores -> ex8 (grouped layout) ----------
            alo8 = mid.tile([128, BLK // 8], dt.float32, tag="alo8")
            for ah in range(2):
                aw = BLK // 16 // 16
                nc.gpsimd.ap_gather(alo8[:, ah * (BLK // 16):
                                         (ah + 1) * (BLK // 16)],
                                    al8r[:],
                                    rloc8[:, ah * aw:(ah + 1) * aw],
                                    channels=128, num_elems=NLOC, d=1,
                                    num_idxs=BLK // 16)
            ex8 = res.tile([128, BLK], dt.bfloat16, tag="ex8")
            for cch in range(16):
                a, b = cch * 1024, (cch + 1) * 1024
                xgb = work.tile([128, 8, 1024], dt.bfloat16, tag="xgb",
                                bufs=1)
                for g in range(8):
                    nc.sync.dma_start(
                        xgb[:, g, :], xg_d[:, g * BLK + a:g * BLK + b])
                vsts = work.tile([128, 1024], dt.bfloat16, tag="vsts",
                                 bufs=1)
                nc.sync.dma_start(vsts[:], vst_d[:, a:b])

                arv = psum.tile([128, 1024], dt.float32, tag="p4k",
                                name="arv")
                for half in (0, 1):
                    for gg in range(4):
                        g = 4 * half + gg
                        sel = ARWt[:, 64 * g:64 * g + 64]
                        for v2 in range(2):
                            va = v2 * 512
                            nc.tensor.matmul(
                                arv[64 * half:64 * half + 64, va:va + 512],
                                sel, xgb[:, g, va:va + 512],
                                start=(gg == 0), stop=(gg == 3))
                p8 = work.tile([128, 1024], dt.bfloat16, tag="b2k",
                               name="p8")
                albc = alo8[:, cch * 128:(cch + 1) * 128].rearrange(
                    "p (a b) -> p a b", b=1).broadcast_to([128, 128, 8])
                arvv = arv[:].rearrange("p (a b) -> p a b", b=8)
                p8v = p8[:].rearrange("p (a b) -> p a b", b=8)
                nc.vector.tensor_tensor(p8v, albc, arvv, AL.add)
                nc.vector.tensor_tensor(p8[:], p8[:], vsts[:], AL.mult)
                sxp = psum.tile([128, 1024], dt.float32, tag="p4k",
                                name="sxp")
                nc.tensor.matmul(sxp[:, 0:512], CSELt[:], p8[:, 0:512],
                                 start=True, stop=True)
                nc.tensor.matmul(sxp[:, 512:1024], CSELt[:],
                                 p8[:, 512:1024], start=True, stop=True)
                nc.scalar.activation(ex8[:, a:b], sxp[:], AF.Exp)

            # ---------- denom: edge scans + pair readout ----------
            exs = res.tile([128, BLK], dt.bfloat16, tag="sc32b",
                           name="exs")
            for sq in range(4):
                a, b = sq * SEG, (sq + 1) * SEG
                emaskh = mid.tile([128, 4096], dt.bfloat16, tag="mask8k",
                                  name="emaskh")
                nc.sync.dma_start(emaskh[:], emrep_d[:, a:b])
                nc.vector.tensor_tensor_scan(
                    exs[:, a:b], emaskh[:], ex8[:, a:b], 0.0,
                    op0=AL.mult, op1=AL.add)
            dendw = mid.tile([128, NLE // 16], dt.int16, tag="endw",
                             bufs=3, name="dendw")
            nc.sync.dma_start(dendw[:], dendp_d[:])
            exsp = exs[:].rearrange("p (a b) -> p a b", b=2)
            for j in range(NJ1K):
                a, b = j * 1024, min(NLE, (j + 1) * 1024)
                dgp = work.tile([128, 1024, 2], dt.bfloat16, tag="g4k",
                                name="dgp")
                nc.gpsimd.ap_gather(
                    dgp[:, :b - a, :], exsp,
                    dendw[:, a // 16:(a + (b - a)) // 16],
                    channels=128, num_elems=BLK // 2, d=2,
                    num_idxs=b - a)
                pdn = psum.tile([2, 1024], dt.float32, tag="pdn",
                                name="pdn", bufs=1)
                for va in range(0, b - a, 512):
                    vb = min(b - a, va + 512)
                    nc.tensor.matmul(pdn[:, va:vb], DSELt[:],
                                     dgp[:, va:vb, 1],
                                     start=True, stop=True)
                nc.vector.tensor_tensor(dsum[:, a:b], dsum[:, a:b],
                                        pdn[:, :b - a], AL.add)

            nc.vector.tensor_scalar(dsum[:], dsum[:], 1e-8, None, AL.add)
            drec = dsum
            nc.vector.reciprocal(drec[:], dsum[:])

            # ---------- C: messages, single pass ----------
            pp = res.tile([128, NPART], dt.bfloat16, tag="sc32a",
                          name="pp")
            for s in range(32):
                # subiter covers 4096 slots = 512 partials
                s0 = s * 4096
                t = s0 // BLK
                e0 = s0 % BLK
                for cc in range(4):
                    c0 = s0 + cc * 1024
                    ce = e0 + cc * 1024
                    xgc = work.tile([128, 1024], dt.bfloat16,
                                    tag="xgc")
                    nc.sync.dma_start(xgc[:], xg_d[:, c0:c0 + 1024])
                    wxe = psum.tile([128, 1024], dt.float32, tag="p4k",
                                    name="wxe")
                    nc.tensor.matmul(wxe[:, 0:512], W01[:],
                                     xgc[:, 0:512],
                                     start=True, stop=True)
                    nc.tensor.matmul(wxe[:, 512:1024], W01[:],
                                     xgc[:, 512:1024],
                                     start=True, stop=True)
                    gtS = work.tile([128, 1024], dt.bfloat16,
                                    tag="b2k", name="gtS")
                    nc.scalar.activation(gtS[:], wxe[:], AF.Copy)
                    exrep = psum.tile([128, 1024], dt.float32,
                                      tag="p4k", name="exrep")
                    sel = SELt[:, t * 128:(t + 1) * 128]
                    nc.tensor.matmul(exrep[:, 0:512], sel,
                                     ex8[:, ce:ce + 512],
                                     start=True, stop=True)
                    nc.tensor.matmul(exrep[:, 512:1024], sel,
                                     ex8[:, ce + 512:ce + 1024],
                                     start=True, stop=True)
                    gb = work.tile([128, 128, 8], dt.bfloat16,
                                   tag="gb")
                    g2o = gb[:].rearrange("p a b -> p (a b)")
                    nc.vector.tensor_tensor(g2o[:, :], gtS[:],
                                            exrep[:], AL.mult)
                    pb = s * 512 + cc * 128
                    nc.vector.tensor_reduce(
                        pp[:, pb:pb + 128], gb[:], axis=AX.X,
                        op=AL.add)
            ppscan = res.tile([128, NPART], dt.bfloat16, tag="sc32b",
                              name="ppscan")
            for sq in range(4):
                a, b = sq * 4096, (sq + 1) * 4096
                pmq = mid.tile([128, 4096], dt.bfloat16, tag="mask8k",
                               name="pmq")
                nc.sync.dma_start(pmq[:], pmrep_d[:, a:b])
                nc.vector.tensor_tensor_scan(
                    ppscan[:, a:b], pmq[:], pp[:, a:b], 0.0,
                    op0=AL.mult, op1=AL.add)
            endw = mid.tile([128, NLE // 16], dt.int16, tag="endw",
                            bufs=3, name="endw")
            nc.sync.dma_start(endw[:], endp_d[:])
            scp = ppscan[:].rearrange("p (a b) -> p a b", b=2)
            msb = res.tile([128, NLOC], dt.bfloat16, tag="sc32b",
                           name="msb")
            xTr = res.tile([128, NLOC], dt.bfloat16, tag="sc32a",
                           name="xTr")
            nc.sync.dma_start(xTr[:], xT_d[:])
            WRTt = mid.tile([128, 64], dt.bfloat16, tag="w128")
            WRBt = mid.tile([128, 64], dt.bfloat16, tag="w128b")
            nc.sync.dma_start(WRTt[:], WRT_d[:])
            nc.sync.dma_start(WRBt[:], WRB_d[:])
            osb = res.tile([64, NLOC], dt.float32, tag="ex8", name="osb")
            for j in range(NJ1K):
                a, b = j * 1024, min(NLE, (j + 1) * 1024)
                ehp = work.tile([128, 1024, 2], dt.bfloat16,
                                tag="g4k", name="ehp")
                nc.gpsimd.ap_gather(
                    ehp[:, :b - a, :], scp,
                    endw[:, a // 16:(a + (b - a)) // 16],
                    channels=128, num_elems=NPART // 2, d=2,
                    num_idxs=b - a)
                parc = work.tile([128, 1024], dt.bfloat16, tag="xgc",
                                 name="parc")
                nc.sync.dma_start(parc[:, :b - a], parw_d[:, a:b])
                pdf = work.tile([128, 1024], dt.bfloat16, tag="b2k",
                                name="pdf")
                nc.vector.tensor_tensor(pdf[:, :b - a],
                                        ehp[:, :b - a, 1],
                                        ehp[:, :b - a, 0], AL.subtract)
                nc.vector.tensor_tensor(pdf[:, :b - a], pdf[:, :b - a],
                                        parc[:, :b - a], AL.mult)
                nc.vector.tensor_tensor(
                    msum[:, a:b], msum[:, a:b],
                    ehp[:, :b - a, 0], AL.add)
                nc.vector.tensor_tensor(
                    msum[:, a:b], msum[:, a:b],
                    pdf[:, :b - a], AL.add)
                # stage-4 for the finished column range, under the next
                # gather's latency
                for a2 in range(a, min(b, NLOC), 512):
                    b2 = min(a2 + 512, NLOC)
                    drep = psum.tile([128, 1024], dt.float32, tag="p4k",
                                     name="drep")
                    nc.tensor.matmul(drep[:, :b2 - a2], DSEL2t[:],
                                     drec[:, a2:b2], start=True, stop=True)
                    nc.vector.tensor_tensor(msb[:, a2:b2],
                                            msum[:, a2:b2],
                                            drep[:, :b2 - a2], AL.mult)
                    prj = psum.tile([64, 1024], dt.float32, tag="pdn",
                                    name="prj", bufs=1)
                    nc.tensor.matmul(prj[:, :b2 - a2], WRTt[:],
                                     xTr[:, a2:b2], start=True, stop=False)
                    nc.tensor.matmul(prj[:, :b2 - a2], WRBt[:],
                                     msb[:, a2:b2], start=False, stop=True)
                    et = work.tile([64, 512], dt.float32, tag="et4k",
                                   bufs=1, name="et")
                    nc.scalar.activation(et[:, :b2 - a2], prj[:, :b2 - a2],
                                         AF.Exp)
                    nc.vector.tensor_scalar(et[:, :b2 - a2],
                                            et[:, :b2 - a2],
                                            -1.0, 0.0, AL.add, AL.min)
                    nc.vector.scalar_tensor_tensor(
                        osb[:, a2:b2], prj[:, :b2 - a2], 0.0,
                        et[:, :b2 - a2], op0=AL.max, op1=AL.add)
            nc.sync.dma_start(out_d[:], osb[:])

    nc.compile()
    return nc


_CACHED = {}


def kernel(**inputs):
    import sys
    if '/opt/trn_rl_repo' not in sys.path:
        sys.path.insert(0, '/opt/trn_rl_repo')
    from concourse import bass_utils

    np_inputs = {k: np.asarray(v) for k, v in inputs.items()}
    in_maps = host_prep(**np_inputs)
    for im in in_maps:
        im.pop("_dbg", None)
    if 'nc' not in _CACHED:
        _CACHED['nc'] = build_bass()
    nc = _CACHED['nc']
    res = bass_utils.run_bass_kernel_spmd(nc, in_maps,
                                          core_ids=list(range(NC)))
    outs = [res.results[m]["out"] for m in range(NC)]
    return np.concatenate([o.T for o in outs], axis=0).astype(np.float32)
